# revision 21
# baseline (speedup 1.0000x reference)
"""Trainium2 Bass kernel for a ClassificationHead:
  h = x[:, 1:, :]                      # drop CLS token
  h = LayerNorm(h) * gamma + beta      # over last dim (768)
  logits = h @ W.T + bias              # W: [1, 768]
  out = sigmoid(logits)                # [256, 256, 1]

Math reformulation (everything becomes per-token reductions over e=768):
  geff = gamma * W[0]
  g2   = geff - sum(geff)/768    # folds the LN mean-correction into the weights
  c    = dot(beta, W[0]) + bias[0]
  s2[t]  = dot(h[t], g2)
  var[t] = population variance of h[t]
  out[t] = sigmoid(s2[t] / sqrt(var[t] + eps) + c)

Sharding: data-parallel over 8 NeuronCores, 32 batches (8192 tokens) per core.
Token-to-column mapping: stat column `col` holds tokens {64*p + col} so the
final [128, 64] result tile stores contiguously to DRAM.

Engine split (balanced so each engine hides under the ~70us/core HBM read):
  - DVE: the g2-dot for every column (scalar_tensor_tensor accum), plus
    bn_stats/bn_aggr (mean+var in one pass) for 3 of every 8 columns, plus
    a couple of plain sums for fine balance.
  - ACT: Square-accum (sum of squares) + Copy-accum (plain sum) for the
    remaining 5 of 8 columns; Sqrt/Sigmoid epilogue. Accumulator outputs
    land in PSUM (ACT sits closer to PSUM; cheaper accumulator drain).
  - Columns are interleaved bn/ACT at period 8 so both engines stream
    concurrently; ACT tables are pre-warmed; the epilogue runs per
    column-half so only the second half sits on the critical-path tail.
"""

import os

import numpy as np

import concourse.bacc as bacc
import concourse.bass as bass
import concourse.tile as tile
from concourse import mybir
from concourse.bass_utils import run_bass_kernel_spmd

B, N, E = 256, 257, 768
N_CORES = 8
BS = B // N_CORES          # batches per core
T = BS * (N - 1)           # tokens per core = 8192
P = 128                    # partitions
S = T // P                 # stat columns per core = 64
EPS = 1e-5

_CACHE = {}
LAST_RESULTS = None        # test harness reads exec_time_ns off this


def _build_nc():
    nc = bacc.Bacc(None, target_bir_lowering=False)
    f32 = mybir.dt.float32
    J = 2                       # columns per DMA
    G = 8                       # column group size for the bn/ACT pattern
    K = 3                       # bn columns per group
    NH = 2                      # epilogue halves
    SH = S // NH                # columns per half = 32
    NGH = SH // G               # groups per half = 4
    n_act = G - K

    x = nc.dram_tensor("x", [T, E], f32, kind="ExternalInput")
    # params: [:, :768] = g2 replicated across partitions, [:, 768] = c
    params = nc.dram_tensor("params", [P, E + 1], f32, kind="ExternalInput")
    out = nc.dram_tensor("out", [T], f32, kind="ExternalOutput")
    # x_rj[s][p, :] = rows {S*p + J*s + j} of x, contiguous per partition
    x_rj = x.ap().rearrange("(p s j) e -> s p (j e)", p=P, j=J)
    out_r = out.ap().rearrange("(p s) -> p s", p=P)

    with tile.TileContext(nc) as tc:
        with (
            tc.tile_pool(name="singles", bufs=1) as singles,
            tc.tile_pool(name="loads", bufs=8) as loads,
            tc.tile_pool(name="work", bufs=3) as work,
            tc.tile_pool(name="stats", bufs=1) as stats_pool,
            tc.tile_pool(name="accums", bufs=1, space="PSUM") as accums,
        ):
            params_t = singles.tile([P, E + 1], f32)
            g2_t = params_t[:, 0:E]
            c_ap = params_t[:, E : E + 1]
            eps_t = singles.tile([P, 1], f32)
            nc.vector.memset(eps_t, EPS)

            # pre-warm the Sqrt/Sigmoid ACT tables so the epilogue doesn't
            # pay two serial ~1.3us lazy table loads
            warm = singles.tile([P, 1], f32)
            nc.scalar.activation(
                out=warm, in_=eps_t,
                func=mybir.ActivationFunctionType.Sqrt, bias=eps_t, scale=1.0,
            )
            nc.scalar.activation(
                out=warm, in_=warm,
                func=mybir.ActivationFunctionType.Sigmoid, bias=0.0, scale=1.0,
            )

            s2 = [
                stats_pool.tile([P, SH], f32, name=f"s2_{h}") for h in range(NH)
            ]
            mv = [
                stats_pool.tile([P, NGH, K, 2], f32, name=f"mv_{h}")
                for h in range(NH)
            ]
            sm = [
                accums.tile([P, NGH, n_act], f32, name=f"sm_{h}")
                for h in range(NH)
            ]
            sq = [
                accums.tile([P, NGH, n_act], f32, name=f"sq_{h}")
                for h in range(NH)
            ]
            # the last two columns run as bn columns on DVE so ACT's accum
            # stream ends early and the epilogue table loads overlap compute
            mvx = stats_pool.tile([P, 2, 2], f32, name="mvx")
            res_all = stats_pool.tile([P, S], f32, name="res_all")

            def epilogue(h):
                # var assembly + mu/musq run on ACT: it drains its accum
                # stream a few us before DVE and would otherwise idle here
                var = stats_pool.tile([P, NGH, G], f32, name=f"var_{h}")
                nc.scalar.activation(
                    out=var[:, :, 0:K], in_=mv[h][:, :, :, 1],
                    func=mybir.ActivationFunctionType.Copy,
                )
                mu = stats_pool.tile([P, NGH, n_act], f32, name=f"mu_{h}")
                nc.scalar.activation(
                    out=mu, in_=sm[h],
                    func=mybir.ActivationFunctionType.Copy, scale=1.0 / E,
                )
                musq = stats_pool.tile([P, NGH, n_act], f32, name=f"musq_{h}")
                nc.scalar.activation(
                    out=musq, in_=mu,
                    func=mybir.ActivationFunctionType.Square,
                )
                nc.vector.scalar_tensor_tensor(
                    out=var[:, :, K:G], in0=sq[h], scalar=1.0 / E, in1=musq,
                    op0=mybir.AluOpType.mult, op1=mybir.AluOpType.subtract,
                )
                if h == 1:
                    # cols 62/63 were bn columns; their act-slot var entries
                    # are garbage from uninitialized accums — overwrite last
                    nc.scalar.activation(
                        out=var[:, 3, 6:8], in_=mvx[:, :, 1],
                        func=mybir.ActivationFunctionType.Copy,
                    )
                varf = var.rearrange("p a b -> p (a b)")
                std = stats_pool.tile([P, SH], f32, name=f"std_{h}")
                nc.scalar.activation(
                    out=std, in_=varf,
                    func=mybir.ActivationFunctionType.Sqrt,
                    bias=eps_t, scale=1.0,
                )
                rstd = stats_pool.tile([P, SH], f32, name=f"rstd_{h}")
                nc.vector.reciprocal(out=rstd, in_=std)
                logit = stats_pool.tile([P, SH], f32, name=f"logit_{h}")
                nc.vector.tensor_mul(out=logit, in0=s2[h], in1=rstd)
                nc.scalar.activation(
                    out=res_all[:, h * SH : (h + 1) * SH], in_=logit,
                    func=mybir.ActivationFunctionType.Sigmoid,
                    bias=c_ap, scale=1.0,
                )
                if h == NH - 1:
                    nc.sync.dma_start(out=out_r, in_=res_all)

            x_r1 = x.ap().rearrange("(p s) e -> s p e", p=P)
            for s in range(S // J):
                if s == 0:
                    # first two columns get individual 393KB loads so DVE's
                    # first op starts one half-transfer earlier
                    x_a = loads.tile([P, E], f32, name="x_a")
                    nc.sync.dma_start(out=x_a, in_=x_r1[0])
                    x_b = loads.tile([P, E], f32, name="x_b")
                    nc.sync.dma_start(out=x_b, in_=x_r1[1])
                    # params gate only the dots (not bn_stats); loading them
                    # after col 0 lets compute start one transfer earlier
                    nc.sync.dma_start(out=params_t, in_=params.ap())
                    first = [x_a, x_b]
                else:
                    x_t = loads.tile([P, J * E], f32)
                    nc.sync.dma_start(out=x_t, in_=x_rj[s])

                for j in range(J):
                    col = J * s + j
                    h, ch = col // SH, col % SH
                    g, i = ch // G, ch % G
                    xj = first[j] if s == 0 else x_t[:, j * E : (j + 1) * E]

                    if i < K or col >= S - 2:
                        # mean+var in one DVE pass (two 384-wide bn_stats)
                        x2 = xj.rearrange("p (w f) -> p w f", w=2)
                        st = work.tile([P, 2, 6], f32, tag="bnstats")
                        for w in range(2):
                            nc.vector.bn_stats(out=st[:, w, :], in_=x2[:, w, :])
                        dst = (
                            mv[h][:, g, i, :] if i < K
                            else mvx[:, col - (S - 2), :]
                        )
                        nc.vector.bn_aggr(out=dst, in_=st)
                    else:
                        ac = i - K
                        d_sq = work.tile([P, 1], f32, tag="d_sq")
                        nc.scalar.activation(
                            out=d_sq.broadcast_to(xj.shape), in_=xj,
                            func=mybir.ActivationFunctionType.Square,
                            accum_out=sq[h][:, g, ac : ac + 1],
                        )
                        d_sm = work.tile([P, 1], f32, tag="d_sm")
                        nc.scalar.activation(
                            out=d_sm.broadcast_to(xj.shape), in_=xj,
                            func=mybir.ActivationFunctionType.Copy,
                            accum_out=sm[h][:, g, ac : ac + 1],
                        )

                    d = work.tile([P, 1], f32, tag="d")
                    nc.vector.scalar_tensor_tensor(
                        out=d.broadcast_to(xj.shape), in0=xj, scalar=1.0,
                        in1=g2_t,
                        op0=mybir.AluOpType.mult, op1=mybir.AluOpType.mult,
                        accum_out=s2[h][:, ch : ch + 1],
                    )

            # both halves at the end: a mid-kernel Sqrt/Sigmoid epilogue
            # thrashes the ACT table cache (two extra 1.3us reloads)
            epilogue(0)
            epilogue(1)

    nc.compile()
    return nc


def kernel(x, ln_gamma, ln_beta, W, bias):
    global LAST_RESULTS
    x = np.ascontiguousarray(np.asarray(x, dtype=np.float32))
    ln_gamma = np.asarray(ln_gamma, dtype=np.float32)
    ln_beta = np.asarray(ln_beta, dtype=np.float32)
    W = np.asarray(W, dtype=np.float32)
    bias = np.asarray(bias, dtype=np.float32)

    geff = ln_gamma * W[0]
    g2 = geff - geff.sum() / E
    c = float(ln_beta @ W[0] + bias[0])

    params = np.empty((P, E + 1), dtype=np.float32)
    params[:, :E] = g2[None, :]
    params[:, E] = c

    # drop CLS, shard over cores, flatten to [T, E] per core
    h = x[:, 1:, :]                                  # [256, 256, 768]
    shards = [
        np.ascontiguousarray(h[i * BS : (i + 1) * BS].reshape(T, E))
        for i in range(N_CORES)
    ]

    if "nc" not in _CACHE:
        _CACHE["nc"] = _build_nc()
    nc = _CACHE["nc"]

    in_maps = [{"x": shards[i], "params": params} for i in range(N_CORES)]
    trace = bool(int(os.environ.get("BASS_KERNEL_TRACE", "0")))
    results = run_bass_kernel_spmd(
        nc, in_maps, core_ids=list(range(N_CORES)), trace=trace
    )
    LAST_RESULTS = results

    outs = [results.results[i]["out"] for i in range(N_CORES)]
    full = np.concatenate(outs).reshape(B, N - 1, 1).astype(np.float32)
    return full


# revision 22
# speedup vs baseline: 1.0308x; 1.0308x over previous
"""Trainium2 Bass kernel for a ClassificationHead:
  h = x[:, 1:, :]                      # drop CLS token
  h = LayerNorm(h) * gamma + beta      # over last dim (768)
  logits = h @ W.T + bias              # W: [1, 768]
  out = sigmoid(logits)                # [256, 256, 1]

Math reformulation (everything becomes per-token reductions over e=768):
  geff = gamma * W[0]
  g2   = geff - sum(geff)/768    # folds the LN mean-correction into the weights
  c    = dot(beta, W[0]) + bias[0]
  s2[t]  = dot(h[t], g2)
  var[t] = population variance of h[t]
  out[t] = sigmoid(s2[t] / sqrt(var[t] + eps) + c)

Sharding: data-parallel over 8 NeuronCores, 32 batches (8192 tokens) per core.
Token-to-column mapping: stat column `col` holds tokens {64*p + col} so the
final [128, 64] result tile stores contiguously to DRAM.

Engine split (balanced so each engine hides under the ~70us/core HBM read):
  - DVE: the g2-dot for every column (scalar_tensor_tensor accum), plus
    bn_stats/bn_aggr (mean+var in one pass) for 3 of every 8 columns, plus
    a couple of plain sums for fine balance.
  - ACT: Square-accum (sum of squares) + Copy-accum (plain sum) for the
    remaining 5 of 8 columns; Sqrt/Sigmoid epilogue. Accumulator outputs
    land in PSUM (ACT sits closer to PSUM; cheaper accumulator drain).
  - Columns are interleaved bn/ACT at period 8 so both engines stream
    concurrently; ACT tables are pre-warmed; the epilogue runs per
    column-half so only the second half sits on the critical-path tail.
"""

import os

import numpy as np

import concourse.bacc as bacc
import concourse.bass as bass
import concourse.tile as tile
from concourse import mybir
from concourse.bass_utils import run_bass_kernel_spmd

B, N, E = 256, 257, 768
N_CORES = 8
BS = B // N_CORES          # batches per core
T = BS * (N - 1)           # tokens per core = 8192
P = 128                    # partitions
S = T // P                 # stat columns per core = 64
EPS = 1e-5

_CACHE = {}
LAST_RESULTS = None        # test harness reads exec_time_ns off this


def _build_nc():
    nc = bacc.Bacc(None, target_bir_lowering=False)
    f32 = mybir.dt.float32
    J = 2                       # columns per DMA
    G = 8                       # column group size for the bn/ACT pattern
    K = 3                       # bn columns per group
    NH = 2                      # epilogue halves
    SH = S // NH                # columns per half = 32
    NGH = SH // G               # groups per half = 4
    n_act = G - K

    x = nc.dram_tensor("x", [T, E], f32, kind="ExternalInput")
    # params: [:, :768] = g2 replicated across partitions, [:, 768] = c
    params = nc.dram_tensor("params", [P, E + 1], f32, kind="ExternalInput")
    out = nc.dram_tensor("out", [T], f32, kind="ExternalOutput")
    # x_rj[s][p, :] = rows {S*p + J*s + j} of x, contiguous per partition
    x_rj = x.ap().rearrange("(p s j) e -> s p (j e)", p=P, j=J)
    out_r = out.ap().rearrange("(p s) -> p s", p=P)

    with tile.TileContext(nc) as tc:
        with (
            tc.tile_pool(name="singles", bufs=1) as singles,
            tc.tile_pool(name="loads", bufs=8) as loads,
            tc.tile_pool(name="work", bufs=3) as work,
            tc.tile_pool(name="stats", bufs=1) as stats_pool,
            tc.tile_pool(name="accums", bufs=1, space="PSUM") as accums,
        ):
            params_t = singles.tile([P, E + 1], f32)
            g2_t = params_t[:, 0:E]
            c_ap = params_t[:, E : E + 1]
            eps_t = singles.tile([P, 1], f32)
            nc.vector.memset(eps_t, EPS)

            # pre-warm the Sqrt/Sigmoid ACT tables so the epilogue doesn't
            # pay two serial ~1.3us lazy table loads
            warm = singles.tile([P, 1], f32)
            nc.scalar.activation(
                out=warm, in_=eps_t,
                func=mybir.ActivationFunctionType.Sqrt, bias=eps_t, scale=1.0,
            )
            nc.scalar.activation(
                out=warm, in_=warm,
                func=mybir.ActivationFunctionType.Sigmoid, bias=0.0, scale=1.0,
            )

            s2 = [
                stats_pool.tile([P, SH], f32, name=f"s2_{h}") for h in range(NH)
            ]
            mv = [
                stats_pool.tile([P, NGH, K, 2], f32, name=f"mv_{h}")
                for h in range(NH)
            ]
            sm = [
                accums.tile([P, NGH, n_act], f32, name=f"sm_{h}")
                for h in range(NH)
            ]
            sq = [
                accums.tile([P, NGH, n_act], f32, name=f"sq_{h}")
                for h in range(NH)
            ]
            # the last two columns run as bn columns on DVE so ACT's accum
            # stream ends early and the epilogue table loads overlap compute
            mvx = stats_pool.tile([P, 2, 2], f32, name="mvx")
            res_all = stats_pool.tile([P, S], f32, name="res_all")

            def epilogue(h):
                # var assembly + mu/musq run on ACT: it drains its accum
                # stream a few us before DVE and would otherwise idle here
                var = stats_pool.tile([P, NGH, G], f32, name=f"var_{h}")
                nc.scalar.activation(
                    out=var[:, :, 0:K], in_=mv[h][:, :, :, 1],
                    func=mybir.ActivationFunctionType.Copy,
                )
                mu = stats_pool.tile([P, NGH, n_act], f32, name=f"mu_{h}")
                nc.scalar.activation(
                    out=mu, in_=sm[h],
                    func=mybir.ActivationFunctionType.Copy, scale=1.0 / E,
                )
                musq = stats_pool.tile([P, NGH, n_act], f32, name=f"musq_{h}")
                nc.scalar.activation(
                    out=musq, in_=mu,
                    func=mybir.ActivationFunctionType.Square,
                )
                nc.vector.scalar_tensor_tensor(
                    out=var[:, :, K:G], in0=sq[h], scalar=1.0 / E, in1=musq,
                    op0=mybir.AluOpType.mult, op1=mybir.AluOpType.subtract,
                )
                if h == 1:
                    # cols 62/63 were bn columns; their act-slot var entries
                    # are garbage from uninitialized accums — overwrite last
                    nc.scalar.activation(
                        out=var[:, 3, 6:8], in_=mvx[:, :, 1],
                        func=mybir.ActivationFunctionType.Copy,
                    )
                varf = var.rearrange("p a b -> p (a b)")
                std = stats_pool.tile([P, SH], f32, name=f"std_{h}")
                nc.scalar.activation(
                    out=std, in_=varf,
                    func=mybir.ActivationFunctionType.Sqrt,
                    bias=eps_t, scale=1.0,
                )
                rstd = stats_pool.tile([P, SH], f32, name=f"rstd_{h}")
                nc.vector.reciprocal(out=rstd, in_=std)
                logit = stats_pool.tile([P, SH], f32, name=f"logit_{h}")
                nc.vector.tensor_mul(out=logit, in0=s2[h], in1=rstd)
                nc.scalar.activation(
                    out=res_all[:, h * SH : (h + 1) * SH], in_=logit,
                    func=mybir.ActivationFunctionType.Sigmoid,
                    bias=c_ap, scale=1.0,
                )
                if h == NH - 1:
                    nc.sync.dma_start(out=out_r, in_=res_all)

            for s in range(S // J):
                x_t = loads.tile([P, J * E], f32)
                nc.sync.dma_start(out=x_t, in_=x_rj[s])
                if s == 0:
                    # params gate only the dots (not bn_stats); loading them
                    # second lets compute start one transfer earlier
                    nc.sync.dma_start(out=params_t, in_=params.ap())

                for j in range(J):
                    col = J * s + j
                    h, ch = col // SH, col % SH
                    g, i = ch // G, ch % G
                    xj = x_t[:, j * E : (j + 1) * E]

                    if i < K or col >= S - 2:
                        # mean+var in one DVE pass (two 384-wide bn_stats)
                        x2 = xj.rearrange("p (w f) -> p w f", w=2)
                        st = work.tile([P, 2, 6], f32, tag="bnstats")
                        for w in range(2):
                            nc.vector.bn_stats(out=st[:, w, :], in_=x2[:, w, :])
                        dst = (
                            mv[h][:, g, i, :] if i < K
                            else mvx[:, col - (S - 2), :]
                        )
                        nc.vector.bn_aggr(out=dst, in_=st)
                    else:
                        ac = i - K
                        d_sq = work.tile([P, 1], f32, tag="d_sq")
                        nc.scalar.activation(
                            out=d_sq.broadcast_to(xj.shape), in_=xj,
                            func=mybir.ActivationFunctionType.Square,
                            accum_out=sq[h][:, g, ac : ac + 1],
                        )
                        d_sm = work.tile([P, 1], f32, tag="d_sm")
                        nc.scalar.activation(
                            out=d_sm.broadcast_to(xj.shape), in_=xj,
                            func=mybir.ActivationFunctionType.Copy,
                            accum_out=sm[h][:, g, ac : ac + 1],
                        )

                    d = work.tile([P, 1], f32, tag="d")
                    nc.vector.scalar_tensor_tensor(
                        out=d.broadcast_to(xj.shape), in0=xj, scalar=1.0,
                        in1=g2_t,
                        op0=mybir.AluOpType.mult, op1=mybir.AluOpType.mult,
                        accum_out=s2[h][:, ch : ch + 1],
                    )

            # both halves at the end: a mid-kernel Sqrt/Sigmoid epilogue
            # thrashes the ACT table cache (two extra 1.3us reloads)
            epilogue(0)
            epilogue(1)

    nc.compile()
    return nc


def kernel(x, ln_gamma, ln_beta, W, bias):
    global LAST_RESULTS
    x = np.ascontiguousarray(np.asarray(x, dtype=np.float32))
    ln_gamma = np.asarray(ln_gamma, dtype=np.float32)
    ln_beta = np.asarray(ln_beta, dtype=np.float32)
    W = np.asarray(W, dtype=np.float32)
    bias = np.asarray(bias, dtype=np.float32)

    geff = ln_gamma * W[0]
    g2 = geff - geff.sum() / E
    c = float(ln_beta @ W[0] + bias[0])

    params = np.empty((P, E + 1), dtype=np.float32)
    params[:, :E] = g2[None, :]
    params[:, E] = c

    # drop CLS, shard over cores, flatten to [T, E] per core
    h = x[:, 1:, :]                                  # [256, 256, 768]
    shards = [
        np.ascontiguousarray(h[i * BS : (i + 1) * BS].reshape(T, E))
        for i in range(N_CORES)
    ]

    if "nc" not in _CACHE:
        _CACHE["nc"] = _build_nc()
    nc = _CACHE["nc"]

    in_maps = [{"x": shards[i], "params": params} for i in range(N_CORES)]
    trace = bool(int(os.environ.get("BASS_KERNEL_TRACE", "0")))
    results = run_bass_kernel_spmd(
        nc, in_maps, core_ids=list(range(N_CORES)), trace=trace
    )
    LAST_RESULTS = results

    outs = [results.results[i]["out"] for i in range(N_CORES)]
    full = np.concatenate(outs).reshape(B, N - 1, 1).astype(np.float32)
    return full
